# revision 51
# baseline (speedup 1.0000x reference)
"""Trainium2 Bass kernel for nn_CCN1D (circulant GNN message passing).

Strategy
--------
The reference gathers receptive fields on a circulant ring graph and runs
per-edge MLPs followed by segment sums.  Because every gathered row's MLP
output depends only on the *source* vertex, the per-edge MLPs (130k / 250k
rows) collapse to per-vertex MLPs (10k rows) plus sliding-window sums along
the ring:

    dense = relu(X @ W1 + b1)                           [N, 128]
    z_f[u]  = relu(relu(dense[u] @ (w0a_lo+w0a_hi)/13) @ w0b)      [N, 64]
    s0_f[v] = sum_{j=0..12} z_f[(v+j) % N]              (window sum)
    z1_f[u] = relu(relu(concat(s0_f[u], z_f[u])/25 @ w1a) @ w1b)
    s1_f[v] = sum_{j=0..24} z1_f[(v+j) % N]
    (reverse branch identical with backward windows)
    logits  = concat(dense, s0f, s1f, s0r, s1r) @ W2 + b2
    out     = log_softmax(logits) * mask

Sharding: vertices are range-partitioned across 8 cores with a 36-vertex
halo on each side (graph/data parallel; weights replicated; no device
collectives needed - the halo makes every core self-sufficient).

On-chip layout is feature-major ([feature partitions, vertex-lane free dim])
so every matmul contracts over partitions, and the window sums become
prefix-scan + shifted-subtract along the free dimension.

v4 redesign (35.5us v2 -> ~21us; HW-profiled against the TimelineSim
cost model):
- fc1 runs as fp8e4m3 DoubleRow matmuls (0.5 cycles/row): 512-contract in
  two passes, xt + fc1 weights quantized host-side.  Halves input DMA too.
- the reverse branch's +12/+24 lane shifts moved from eviction offsets into
  the *rhs read offset* of the next matmul (t1rs/t2rs persist in full-width
  T1R/T2R tiles).  Both branches' z land in one 128-partition PSUM tile and
  ONE eviction produces the packed Z (and Z1) - 6 fewer evictions; the j=0
  edge columns are zeroed by a tiny matmul against a zero weight block.
- DMA dispatch costs ~1.25us of issuing-SEQ time + ~0.9us completion
  semaphore REGARDLESS of size, so everything is batched: 3 input DMAs
  (wpack incl. fp8 weights via bitcast region, two xt halves on the SP and
  Act HWDGE queues) and 2 output DMAs, split so next-iteration inputs are
  never queued behind this iteration's outputs.
- For_i hardware loops drain ALL engines at the back edge (all-engine
  barrier in the semaphore-reset block), so the timed body unrolls 32
  emits per iteration; double-buffered pools let consecutive emits
  pipeline (steady-state period ~14us in-sim, ~21us on HW).
- output via 125-lane transposes: 10 tiles x 125 owned lanes = 1250 rows,
  clean strided DMAs, no ragged tail; softmax in 2 groups (9+1 tiles) so
  the bulk runs under the last fc2 chunk.
- element-wise work balanced across Act (D/Z/t1s/t2s/one Z1 eviction,
  exp/ln, fc2 bias) / DVE (scans, sub13, t1r/t2r/two Z1 evictions,
  reduce, logit-sub) / Pool (sub25, mask-mul; Pool cannot touch PSUM and
  its ISA subset is TensorTensor/Memset, measured ~0.42-0.6 efficiency).
- PE clock p-state is ramped by 4 warm-up groups per emit (the HW HAM
  clock gate throttles an idle PE; measured faster WITH warm-up even in
  the pipelined loop).
"""

import sys

import numpy as np

for _p in ("/opt/trn_rl_repo",):
    if _p not in sys.path:
        sys.path.insert(0, _p)

N = 10000
NCORES = 8
BLK = N // NCORES          # 1250 vertices per core
HALO = 36                  # 12 (layer-0 window) + 24 (layer-1 window)
W = 1344                   # on-chip free width (1322 valid + pad)
NT = 10                    # output transpose tiles
TP = 125                   # lanes per transpose tile (10*125 = 1250 owned)
RF1, RF2 = 13, 25
C_IN, C_HID, MLP_H, MSG, NCLS = 512, 128, 128, 64, 16
LO, HI = HALO, HALO + BLK  # valid output lane range [36, 1286)
WPACK_COLS = 1676          # bf16 weights/biases/mask + fp8 fc1 + zeros
FP8_FC1 = True             # fc1 via fp8e4m3 DoubleRow (else bf16 4-pass)
FP8_SW, FP8_SX = 16.0, 8.0  # quant scales (avoid fp8 subnormals)
WARM_GROUPS = 4            # PE clock-ramp dummy groups during the DMA phase

CTS0 = ((0, 500), (500, 500), (1000, 344))       # fc1 + layer-0 col tiles
SUB13 = ((1, 488), (488, 988), (988, 1332))      # S0 = P13[+12] - P13[-1]
CTS1 = ((0, 488), (488, 500), (988, 344))        # layer-1 col tiles
SUB25 = ((1, 464), (464, 964), (964, 1286))      # S1 = P25[+24] - P25[-1]
CTS2 = ((36, 411), (411, 911), (911, 1161), (1161, 1286))  # fc2 chunks
TTILES = ((0, 3), (3, 7), (7, 9), (9, 10))       # transpose tiles per chunk

_F32 = np.float32


# --------------------------------------------------------------------------
# structure check (is the input the circulant graph the kernel was built for?)
# --------------------------------------------------------------------------

def _expected_idx():
    v = np.arange(N)
    return {
        "f_rf1": ((v[:, None] + np.arange(RF1)) % N).reshape(-1),
        "f_rf2": ((v[:, None] + np.arange(RF2)) % N).reshape(-1),
        "r_rf1": ((v[:, None] - np.arange(RF1)) % N).reshape(-1),
        "r_rf2": ((v[:, None] - np.arange(RF2)) % N).reshape(-1),
        "own1": np.repeat(v, RF1),
        "own2": np.repeat(v, RF2),
        "self1": v * RF1,
    }


def _structure_matches(inputs):
    try:
        if inputs["sparse_feature"].shape != (N, C_IN):
            return False
        for k, exp in _expected_idx().items():
            got = np.asarray(inputs[k])
            if got.shape != exp.shape or not np.array_equal(got, exp):
                return False
        return True
    except Exception:
        return False


# --------------------------------------------------------------------------
# generic numpy fallback (exact reference semantics, any index content)
# --------------------------------------------------------------------------

def _segment_sum(data, seg, num):
    out = np.zeros((num,) + data.shape[1:], dtype=data.dtype)
    np.add.at(out, seg, data)
    return out


def _np_branch(dense, rf1, rf2, own1, own2, self1, w0a, w0b, w1a, w1b):
    sizes1 = _segment_sum(np.ones(own1.shape, dense.dtype), own1, N)
    sizes2 = _segment_sum(np.ones(own2.shape, dense.dtype), own2, N)
    g = dense[rf1]
    m0 = np.concatenate([g, g], axis=-1) / sizes1[own1][:, None]
    h0 = np.maximum(np.maximum(m0 @ w0a, 0.0) @ w0b, 0.0)
    s0 = _segment_sum(h0, own1, N)
    selfr = h0[self1]
    m1 = np.concatenate([s0[rf2], selfr[rf2]], axis=-1) / sizes2[own2][:, None]
    h1 = np.maximum(np.maximum(m1 @ w1a, 0.0) @ w1b, 0.0)
    s1 = _segment_sum(h1, own2, N)
    return s0, s1


def _reference_numpy(inputs):
    f = {k: np.asarray(v) for k, v in inputs.items()}
    dense = np.maximum(
        f["sparse_feature"].astype(_F32) @ f["fc1_w"] + f["fc1_b"], 0.0
    )
    s0f, s1f = _np_branch(dense, f["f_rf1"], f["f_rf2"], f["own1"], f["own2"],
                          f["self1"], f["mw0a"], f["mw0b"], f["mw1a"], f["mw1b"])
    s0r, s1r = _np_branch(dense, f["r_rf1"], f["r_rf2"], f["own1"], f["own2"],
                          f["self1"], f["rw0a"], f["rw0b"], f["rw1a"], f["rw1b"])
    total = np.concatenate([dense, s0f, s1f, s0r, s1r], axis=1)
    logits = total @ f["fc2_w"] + f["fc2_b"]
    m = logits.max(axis=-1, keepdims=True)
    lse = m + np.log(np.exp(logits - m).sum(axis=-1, keepdims=True))
    return ((logits - lse) * f["mask"][:, None].astype(_F32)).astype(_F32)


# --------------------------------------------------------------------------
# device kernel
# --------------------------------------------------------------------------

_NC = None


def _build_nc(repeat=1, hw_loop=0):
    import concourse.bass as bass
    import concourse.tile as tile
    from concourse import bacc, mybir
    from contextlib import ExitStack

    f32 = mybir.dt.float32
    f32r = mybir.dt.float32r
    bf16 = mybir.dt.bfloat16
    fp8 = mybir.dt.float8e4
    AF = mybir.ActivationFunctionType
    OP = mybir.AluOpType
    DR = mybir.MatmulPerfMode.DoubleRow

    nc = bacc.Bacc(trn_type="TRN2", debug=False)

    xdt = fp8 if FP8_FC1 else bf16
    xt_d = nc.dram_tensor("xt", [128, 4, W], xdt, kind="ExternalInput").ap()
    wpack_d = nc.dram_tensor("wpack", [128, WPACK_COLS], bf16,
                             kind="ExternalInput").ap()
    out_d = nc.dram_tensor("out", [BLK, NCLS], f32, kind="ExternalOutput").ap()

    with tile.TileContext(nc) as tc:
        with ExitStack() as ctx:
            cp = ctx.enter_context(tc.tile_pool(name="consts", bufs=2))
            ap_ = ctx.enter_context(tc.tile_pool(name="acts", bufs=3))
            sp = ctx.enter_context(tc.tile_pool(name="scr", bufs=6))
            pmm = ctx.enter_context(tc.tile_pool(name="pmm", bufs=3, space="PSUM"))
            pzz = ctx.enter_context(tc.tile_pool(name="pzz", bufs=2, space="PSUM"))
            pl = ctx.enter_context(tc.tile_pool(name="pl", bufs=2, space="PSUM"))
            pt = ctx.enter_context(tc.tile_pool(name="pt", bufs=1, space="PSUM"))

            def emit():
                # ---- input DMAs: 3 total (dispatch costs ~1.25us each,
                # so batching beats per-tile chunking)
                xt_pack = cp.tile([128, 4, W], xdt, tag="xtp", name="xt_pack")
                wpack = cp.tile([128, WPACK_COLS], bf16, tag="wpack",
                                name="wpack")
                nc.sync.dma_start(out=wpack, in_=wpack_d)
                nc.sync.dma_start(out=xt_pack[:, 0:2, :], in_=xt_d[:, 0:2, :])
                nc.scalar.dma_start(out=xt_pack[:, 2:4, :],
                                    in_=xt_d[:, 2:4, :])

                if WARM_GROUPS:
                    # clock-ramp groups: read the (already loaded) weight
                    # pack directly - no scratch memset needed.
                    warm = pl.tile([NCLS, 512], f32, tag="psL", name="warm")
                    for i in range(WARM_GROUPS):
                        nc.tensor.matmul(warm, wpack[:, 0:NCLS],
                                         wpack[:, 0:512],
                                         start=(i == 0),
                                         stop=(i == WARM_GROUPS - 1),
                                         skip_group_check=True)

                wfc1 = [wpack[:, 128 * k:128 * (k + 1)] for k in range(4)]

                def dr_lhsT(g):
                    # fc1 fp8 weights live in wpack cols [1356+128g, 1356+
                    # 128(g+1)) (bf16 cols = 256 fp8 each); DoubleRow wants
                    # the [K, 2, M] view.
                    sl = wpack[:, 1356 + 128 * g:1356 + 128 * (g + 1)] \
                        .bitcast(fp8)
                    return bass.AP(tensor=sl.tensor, offset=sl.offset,
                                   ap=[sl.ap[0], [128, 2], [1, 128]])

                wz = {"f": wpack[:, 512:640], "r": wpack[:, 640:768]}
                wzb = {"f": wpack[:, 768:832], "r": wpack[:, 832:896]}
                # layer-1 weight halves: cols 896:1024 hold the s0 (lo) half,
                # cols 1024:1152 the z (hi) half; partitions 0:64 = forward
                # branch, 64:128 = reverse branch (matching S0/Z layout).
                wz1lo = {"f": wpack[0:64, 896:1024],
                         "r": wpack[64:128, 896:1024]}
                wz1hi = {"f": wpack[0:64, 1024:1152],
                         "r": wpack[64:128, 1024:1152]}
                wz1b = {"f": wpack[:, 1152:1216], "r": wpack[:, 1216:1280]}
                w2d = wpack[:, 1280:1296]
                w2s0 = wpack[:, 1296:1312]
                w2s1 = wpack[:, 1312:1328]
                zw64 = wpack[:, 1612:1676]    # 64 zero cols (edge zeroing)
                ident = wpack[0:16, 1328:1344]
                bfc1 = wpack[:, 1344:1345]
                bfc2 = wpack[0:16, 1345:1346]
                maskv = wpack[:, 1346:1346 + NT]

                # ---- persistent activation tiles (bf16; prefix sums f32) ----
                D = ap_.tile([128, W], bf16, tag="D")
                Z = ap_.tile([128, W], bf16, tag="Z")    # [0:64]=z_f, [64:]=z_r>>12
                Z1 = ap_.tile([128, W], bf16, tag="Z1")  # [0:64]=z1_f, [64:]=z1_r>>24
                S0 = ap_.tile([128, W], bf16, tag="S0")  # [0:64]=s0f, [64:]=s0r
                T1R = ap_.tile([128, W], bf16, tag="T1R")  # relu(t1) reverse
                T2R = ap_.tile([128, W], bf16, tag="T2R")  # relu(t2) reverse
                P13 = ap_.tile([128, W], f32, tag="P13")  # prefix sums of Z
                P25 = ap_.tile([128, W], f32, tag="P25")  # prefix sums of Z1
                S1 = ap_.tile([128, W], bf16, tag="S1")  # [0:64]=s1f, [64:]=s1r
                Lsb = ap_.tile([NCLS, W], bf16, tag="Lsb")
                LT = ap_.tile([128, NT, NCLS], f32, tag="LT")
                se = sp.tile([128, NT], f32, tag="se", name="se")
                ex = sp.tile([128, NT, NCLS], f32, tag="ex", name="ex")
                psT = pt.tile([128, NT, NCLS], bf16, tag="psT", name="psT")

                def stage_a(j):
                    """fc1 + layer-0 MLPs + chained scan + window sub."""
                    s, w = CTS0[j]
                    psA = pmm.tile([128, 512], f32, tag="mm", name="psA")
                    if FP8_FC1:
                        for g in range(2):
                            nc.tensor.matmul(psA[:, :w], dr_lhsT(g),
                                             xt_pack[:, 2 * g:2 * g + 2, s:s + w],
                                             start=(g == 0), stop=(g == 1),
                                             perf_mode=DR)
                    else:
                        for k in range(4):
                            nc.tensor.matmul(psA[:, :w], wfc1[k],
                                             xt_pack[:, k, s:s + w],
                                             start=(k == 0), stop=(k == 3))
                    nc.scalar.activation(D[:, s:s + w], psA[:, :w], AF.Relu,
                                         bias=bfc1,
                                         scale=(1.0 / (FP8_SW * FP8_SX)
                                                if FP8_FC1 else 1.0))
                    # forward branch t1
                    t1 = pmm.tile([128, 512], f32, tag="mm", name="t1")
                    nc.tensor.matmul(t1[:, :w], wz["f"], D[:, s:s + w],
                                     start=True, stop=True)
                    t1s = sp.tile([128, 512], bf16, tag="t1s", name="t1sf")
                    nc.scalar.activation(t1s[:, :w], t1[:, :w], AF.Relu)
                    # reverse branch t1 (persistent, consumed shifted by zpr)
                    t1r = pmm.tile([128, 512], f32, tag="mm", name="t1r")
                    nc.tensor.matmul(t1r[:, :w], wz["r"], D[:, s:s + w],
                                     start=True, stop=True)
                    nc.vector.tensor_scalar_max(T1R[:, s:s + w], t1r[:, :w], 0.0)
                    # both branches' z into one PSUM tile; r shifted +12 via
                    # its rhs read offset
                    psZ = pzz.tile([128, 512], f32, tag="zz", name="psZ")
                    nc.tensor.matmul(psZ[0:64, 0:w], wzb["f"], t1s[:, :w],
                                     start=True, stop=True)
                    if j == 0:
                        nc.tensor.matmul(psZ[64:128, 0:12], zw64,
                                         wpack[:, 0:12], start=True, stop=True)
                        nc.tensor.matmul(psZ[64:128, 12:w], wzb["r"],
                                         T1R[:, 0:w - 12], start=True,
                                         stop=True)
                    else:
                        nc.tensor.matmul(psZ[64:128, 0:w], wzb["r"],
                                         T1R[:, s - 12:s + w - 12],
                                         start=True, stop=True)
                    nc.scalar.activation(Z[:, s:s + w], psZ[:, 0:w], AF.Relu)
                    # chained prefix scan over both branches, then the
                    # staircase window-subtract for S0
                    nc.vector.tensor_tensor_scan(
                        P13[:, s:s + w], Z[:, s:s + w], Z[:, s:s + w],
                        initial=(0.0 if s == 0 else P13[:, s - 1:s]),
                        op0=OP.add, op1=OP.bypass)
                    lo, hi = SUB13[j]
                    nc.vector.tensor_sub(S0[:, lo:hi], P13[:, lo + 12:hi + 12],
                                         P13[:, lo - 1:hi - 1])
                    if j == 0:
                        nc.scalar.copy(S0[:, 0:1], P13[:, 12:13])

                def stage_b(j):
                    """layer-1 MLPs + chained scan + window sub (Pool)."""
                    a, w1 = CTS1[j]
                    t2 = pmm.tile([128, 512], f32, tag="mm", name="t2")
                    nc.tensor.matmul(t2[:, :w1], wz1lo["f"], S0[0:64, a:a + w1],
                                     start=True, stop=False)
                    nc.tensor.matmul(t2[:, :w1], wz1hi["f"], Z[0:64, a:a + w1],
                                     start=False, stop=True)
                    t2s = sp.tile([128, 512], bf16, tag="t1s", name="t2sf")
                    nc.scalar.activation(t2s[:, :w1], t2[:, :w1], AF.Relu)
                    t2r = pmm.tile([128, 512], f32, tag="mm", name="t2r")
                    nc.tensor.matmul(t2r[:, :w1], wz1lo["r"], S0[64:128, a:a + w1],
                                     start=True, stop=False)
                    nc.tensor.matmul(t2r[:, :w1], wz1hi["r"],
                                     Z[64:128, a + 12:a + 12 + w1],
                                     start=False, stop=True)
                    nc.vector.tensor_scalar_max(T2R[:, a:a + w1], t2r[:, :w1], 0.0)
                    psZ1 = pzz.tile([128, 512], f32, tag="zz", name="psZ1")
                    nc.tensor.matmul(psZ1[0:64, 0:w1], wz1b["f"], t2s[:, :w1],
                                     start=True, stop=True)
                    if j == 0:
                        nc.tensor.matmul(psZ1[64:128, 0:24], zw64,
                                         wpack[:, 0:24], start=True, stop=True)
                        nc.tensor.matmul(psZ1[64:128, 24:w1], wz1b["r"],
                                         T2R[:, 0:w1 - 24], start=True,
                                         stop=True)
                    else:
                        nc.tensor.matmul(psZ1[64:128, 0:w1], wz1b["r"],
                                         T2R[:, a - 24:a + w1 - 24],
                                         start=True, stop=True)
                    if j == 1:
                        nc.scalar.activation(Z1[:, a:a + w1], psZ1[:, 0:w1],
                                             AF.Relu)
                    else:
                        nc.vector.tensor_scalar_max(Z1[:, a:a + w1],
                                                    psZ1[:, 0:w1], 0.0)
                    nc.vector.tensor_tensor_scan(
                        P25[:, a:a + w1], Z1[:, a:a + w1], Z1[:, a:a + w1],
                        initial=(0.0 if a == 0 else P25[:, a - 1:a]),
                        op0=OP.add, op1=OP.bypass)
                    lo, hi = SUB25[j]
                    eng = nc.vector if j == 2 else nc.gpsimd
                    eng.tensor_sub(S1[:, lo:hi], P25[:, lo + 24:hi + 24],
                                   P25[:, lo - 1:hi - 1])

                def bcast(t2d, n):
                    return bass.AP(tensor=t2d.tensor, offset=t2d.offset,
                                   ap=[t2d.ap[0], [t2d.ap[1][0], n], [0, NCLS]])

                def stage_c(c):
                    """fc2 chunk (s1 straight from P25) + bias + transposes."""
                    lo, hi = CTS2[c]
                    w2w = hi - lo
                    psl = pl.tile([NCLS, 512], f32, tag="psL", name="psl")
                    nc.tensor.matmul(psl[:, :w2w], w2d, D[:, lo:hi],
                                     start=True, stop=False)
                    nc.tensor.matmul(psl[:, :w2w], w2s0, S0[:, lo:hi],
                                     start=False, stop=False)
                    nc.tensor.matmul(psl[:, :w2w], w2s1, S1[:, lo:hi],
                                     start=False, stop=True)
                    nc.scalar.activation(Lsb[:, lo:hi], psl[:, :w2w],
                                         AF.Identity, bias=bfc2)
                    t0, t1_ = TTILES[c]
                    for t in range(t0, t1_):
                        off = LO + TP * t
                        nc.tensor.transpose(psT[0:TP, t, :],
                                            Lsb[:, off:off + TP], ident)

                def softmax_out(t0, t1_, c):
                    """log-softmax + mask + output DMA for transpose tiles
                    [t0, t1).  Logits are bounded (|L| ~ 2): exp without
                    max-subtract."""
                    nt = t1_ - t0
                    lt = LT[0:TP, t0:t1_, :]
                    ps3 = psT[0:TP, t0:t1_, :]
                    seh = se[0:TP, t0:t1_]
                    if nt == 1:
                        # single tile: exp's accumulator IS the class sum
                        nc.scalar.activation(ex[0:TP, t0:t1_, :], ps3, AF.Exp,
                                             accum_out=seh)
                    else:
                        nc.scalar.activation(ex[0:TP, t0:t1_, :], ps3, AF.Exp)
                        nc.vector.reduce_sum(seh, ex[0:TP, t0:t1_, :],
                                             axis=mybir.AxisListType.X)
                    nc.scalar.activation(seh, seh, AF.Ln)
                    nc.vector.tensor_sub(lt, ps3, bcast(seh, nt))
                    mm = nc.vector if nt == 1 else nc.gpsimd
                    mm.tensor_mul(lt, lt, bcast(maskv[0:TP, t0:t1_], nt))

                # interleave stages so PE never starves on eviction chains
                stage_a(0)
                stage_a(1)
                stage_b(0)
                stage_a(2)
                stage_b(1)
                stage_c(0)
                stage_b(2)
                stage_c(1)
                stage_c(2)
                softmax_out(0, 9, 0)
                od = out_d.rearrange("(t p) c -> p t c", p=TP)
                nc.sync.dma_start(out=od[:, 0:9, :], in_=LT[0:TP, 0:9, :])
                stage_c(3)
                softmax_out(9, 10, 1)
                nc.scalar.dma_start(out=od[:, 9:10, :], in_=LT[0:TP, 9:10, :])

            if hw_loop:
                unroll = 32 if hw_loop % 32 == 0 else 2
                assert hw_loop % unroll == 0
                with tc.For_i(0, hw_loop // unroll):
                    for _u in range(unroll):
                        emit()
            else:
                for _rep in range(repeat):
                    emit()

    # Steer the ACT-table pass to natural_log_exp_and_others (covers Relu,
    # Identity, Copy, Exp AND Ln) so the kernel pays one table load instead
    # of a ~2.7us mid-kernel switch before the final Ln.
    import concourse.bacc as bacc_mod
    from concourse import mybir as _mb

    AF = _mb.ActivationFunctionType
    orig_tables = bacc_mod.get_activation_tables
    mine = {AF.Relu, AF.Identity, AF.Copy, AF.Exp, AF.Ln}

    def steered(arch):
        t = orig_tables(arch)
        out = {}
        seen_pref = False
        for name, fns in t.items():
            if name == "natural_log_exp_and_others":
                seen_pref = True
                out[name] = fns
            elif not seen_pref:
                out[name] = type(fns)(f for f in fns if f not in mine)
            else:
                out[name] = fns
        return out

    bacc_mod.get_activation_tables = steered
    try:
        nc.compile()
    finally:
        bacc_mod.get_activation_tables = orig_tables
    return nc


def _get_nc(repeat=1, hw_loop=0):
    global _NC
    if repeat != 1 or hw_loop:
        return _build_nc(repeat, hw_loop)
    if _NC is None:
        _NC = _build_nc()
    return _NC


# --------------------------------------------------------------------------
# host-side sharding + entry point
# --------------------------------------------------------------------------

def _make_in_maps(inputs):
    from concourse import mybir

    bf16np = mybir.dt.np(mybir.dt.bfloat16)
    fp8np = mybir.dt.np(mybir.dt.float8e4)
    sf = np.ascontiguousarray(np.asarray(inputs["sparse_feature"], dtype=_F32))
    maskf = np.asarray(inputs["mask"]).astype(_F32)

    def f(k):
        return np.asarray(inputs[k], dtype=_F32)

    mw0a, rw0a = f("mw0a"), f("rw0a")
    wpack = np.zeros((128, WPACK_COLS), dtype=_F32)
    wpack[:, 0:512] = f("fc1_w").reshape(4, 128, C_HID).transpose(1, 0, 2) \
        .reshape(128, 512)
    wpack[:, 512:640] = (mw0a[:C_HID] + mw0a[C_HID:]) / RF1
    wpack[:, 640:768] = (rw0a[:C_HID] + rw0a[C_HID:]) / RF1
    wpack[:, 768:832] = f("mw0b")
    wpack[:, 832:896] = f("rw0b")
    mw1a, rw1a = f("mw1a") / RF2, f("rw1a") / RF2
    wpack[0:64, 896:1024] = mw1a[0:64]      # s0 half, forward
    wpack[64:128, 896:1024] = rw1a[0:64]    # s0 half, reverse
    wpack[0:64, 1024:1152] = mw1a[64:128]   # z half, forward
    wpack[64:128, 1024:1152] = rw1a[64:128]  # z half, reverse
    wpack[:, 1152:1216] = f("mw1b")
    wpack[:, 1216:1280] = f("rw1b")
    w2 = f("fc2_w")
    wpack[:, 1280:1296] = w2[0:128]         # dense
    wpack[0:64, 1296:1312] = w2[128:192]    # s0f
    wpack[64:128, 1296:1312] = w2[256:320]  # s0r
    wpack[0:64, 1312:1328] = w2[192:256]    # s1f
    wpack[64:128, 1312:1328] = w2[320:384]  # s1r
    wpack[0:16, 1328:1344] = np.eye(NCLS, dtype=_F32)
    wpack[:, 1344] = f("fc1_b")
    wpack[0:NCLS, 1345] = f("fc2_b")

    # fc1 weights as [128, k, 128] blocks (k = input-channel block),
    # scaled into fp8's normal range (descaled in the D eviction)
    wf8 = np.ascontiguousarray(
        f("fc1_w").reshape(4, 128, C_HID).transpose(1, 0, 2)
        * FP8_SW).astype(fp8np)

    in_maps = []
    for c in range(NCORES):
        b = c * BLK
        idx = (b - HALO + np.arange(W)) % N
        xt2 = np.ascontiguousarray(sf[idx].T)          # [512, W]
        if FP8_FC1:
            xt2 = xt2 * FP8_SX
        xt = np.ascontiguousarray(
            xt2.reshape(4, 128, W).transpose(1, 0, 2))  # [128, 4, W]
        me = np.zeros(128 * NT, dtype=_F32)
        me[:BLK] = maskf[(b + np.arange(BLK)) % N]
        wc = wpack.copy()
        wc[0:TP, 1346:1346 + NT] = me[:BLK].reshape(NT, TP).T
        wcb = wc.astype(bf16np)
        if FP8_FC1:
            wcb.view(np.uint8).reshape(128, 2 * WPACK_COLS)[:, 2712:3224] = \
                wf8.view(np.uint8).reshape(128, 512)
        m = {"wpack": wcb,
             "xt": xt.astype(fp8np if FP8_FC1 else bf16np)}
        in_maps.append(m)
    return in_maps


_RUNNER = None


def _make_runner():
    """Build the 8-core PJRT executor once; reuse across kernel() calls."""
    import jax
    from jax.sharding import Mesh, NamedSharding, PartitionSpec
    from jax.experimental.shard_map import shard_map
    from concourse import mybir
    from concourse.bass2jax import (_bass_exec_p, install_neuronx_cc_hook,
                                    partition_id_tensor)

    nc = _get_nc()
    install_neuronx_cc_hook()
    in_names, out_names, out_avals, zero_shapes = [], [], [], []
    pname = nc.partition_id_tensor.name if nc.partition_id_tensor else None
    for alloc in nc.m.functions[0].allocations:
        if not isinstance(alloc, mybir.MemoryLocationSet):
            continue
        name = alloc.memorylocations[0].name
        if alloc.kind == "ExternalInput":
            if name != pname:
                in_names.append(name)
        elif alloc.kind == "ExternalOutput":
            out_names.append(name)
            shape = tuple(alloc.tensor_shape)
            dtype = mybir.dt.np(alloc.dtype)
            out_avals.append(jax.core.ShapedArray(shape, dtype))
            zero_shapes.append((shape, dtype))
    n_params = len(in_names)
    all_in = list(in_names) + list(out_names)
    if pname is not None:
        all_in.append(pname)
    donate = tuple(range(n_params, n_params + len(out_names)))

    def _body(*args):
        operands = list(args)
        if pname is not None:
            operands.append(partition_id_tensor())
        return tuple(_bass_exec_p.bind(
            *operands,
            out_avals=tuple(out_avals),
            in_names=tuple(all_in),
            out_names=tuple(out_names),
            lowering_input_output_aliases=(),
            sim_require_finite=True,
            sim_require_nnan=True,
            nc=nc,
        ))

    devices = jax.devices()[:NCORES]
    mesh = Mesh(np.asarray(devices), ("core",))
    shd = NamedSharding(mesh, PartitionSpec("core"))
    n_outs = len(out_names)
    sharded = jax.jit(
        shard_map(_body, mesh=mesh,
                  in_specs=(PartitionSpec("core"),) * (n_params + n_outs),
                  out_specs=(PartitionSpec("core"),) * n_outs,
                  check_rep=False),
        donate_argnums=donate, keep_unused=True,
    )

    def run(in_maps):
        concat_in = [
            np.concatenate([np.asarray(in_maps[c][nm]) for c in range(NCORES)],
                           axis=0)
            for nm in in_names
        ]
        dev_in = [jax.device_put(x, shd) for x in concat_in]
        zeros = [
            jax.device_put(np.zeros((NCORES * s[0], *s[1:]), dt), shd)
            for s, dt in zero_shapes
        ]
        outs = sharded(*dev_in, *zeros)
        res = np.asarray(outs[out_names.index("out")])
        return np.ascontiguousarray(res.reshape(NCORES * BLK, NCLS))

    return run


def kernel(**inputs):
    if not _structure_matches(inputs):
        return _reference_numpy(inputs)
    global _RUNNER
    if _RUNNER is None:
        _RUNNER = _make_runner()
    return _RUNNER(_make_in_maps(inputs))


# revision 53
# speedup vs baseline: 1.0033x; 1.0033x over previous
"""Trainium2 Bass kernel for nn_CCN1D (circulant GNN message passing).

Strategy
--------
The reference gathers receptive fields on a circulant ring graph and runs
per-edge MLPs followed by segment sums.  Because every gathered row's MLP
output depends only on the *source* vertex, the per-edge MLPs (130k / 250k
rows) collapse to per-vertex MLPs (10k rows) plus sliding-window sums along
the ring:

    dense = relu(X @ W1 + b1)                           [N, 128]
    z_f[u]  = relu(relu(dense[u] @ (w0a_lo+w0a_hi)/13) @ w0b)      [N, 64]
    s0_f[v] = sum_{j=0..12} z_f[(v+j) % N]              (window sum)
    z1_f[u] = relu(relu(concat(s0_f[u], z_f[u])/25 @ w1a) @ w1b)
    s1_f[v] = sum_{j=0..24} z1_f[(v+j) % N]
    (reverse branch identical with backward windows)
    logits  = concat(dense, s0f, s1f, s0r, s1r) @ W2 + b2
    out     = log_softmax(logits) * mask

Sharding: vertices are range-partitioned across 8 cores with a 36-vertex
halo on each side (graph/data parallel; weights replicated; no device
collectives needed - the halo makes every core self-sufficient).

On-chip layout is feature-major ([feature partitions, vertex-lane free dim])
so every matmul contracts over partitions, and the window sums become
prefix-scan + shifted-subtract along the free dimension.

v4 redesign (35.5us v2 -> ~21us; HW-profiled against the TimelineSim
cost model):
- fc1 runs as fp8e4m3 DoubleRow matmuls (0.5 cycles/row): 512-contract in
  two passes, xt + fc1 weights quantized host-side.  Halves input DMA too.
- the reverse branch's +12/+24 lane shifts moved from eviction offsets into
  the *rhs read offset* of the next matmul (t1rs/t2rs persist in full-width
  T1R/T2R tiles).  Both branches' z land in one 128-partition PSUM tile and
  ONE eviction produces the packed Z (and Z1) - 6 fewer evictions; the j=0
  edge columns are zeroed by a tiny matmul against a zero weight block.
- DMA dispatch costs ~1.25us of issuing-SEQ time + ~0.9us completion
  semaphore REGARDLESS of size, so everything is batched: 3 input DMAs
  (wpack incl. fp8 weights via bitcast region, two xt halves on the SP and
  Act HWDGE queues) and 2 output DMAs, split so next-iteration inputs are
  never queued behind this iteration's outputs.
- For_i hardware loops drain ALL engines at the back edge (all-engine
  barrier in the semaphore-reset block), so the timed body unrolls 32
  emits per iteration; double-buffered pools let consecutive emits
  pipeline (steady-state period ~14us in-sim, ~21us on HW).
- output via 125-lane transposes: 10 tiles x 125 owned lanes = 1250 rows,
  clean strided DMAs, no ragged tail; softmax in 2 groups (9+1 tiles) so
  the bulk runs under the last fc2 chunk.
- element-wise work balanced across Act (D/Z/t1s/t2s/one Z1 eviction,
  exp/ln, fc2 bias) / DVE (scans, sub13, t1r/t2r/two Z1 evictions,
  reduce, logit-sub) / Pool (sub25, mask-mul; Pool cannot touch PSUM and
  its ISA subset is TensorTensor/Memset, measured ~0.42-0.6 efficiency).
- PE clock p-state is ramped by 4 warm-up groups per emit (the HW HAM
  clock gate throttles an idle PE; measured faster WITH warm-up even in
  the pipelined loop).
"""

import sys

import numpy as np

for _p in ("/opt/trn_rl_repo",):
    if _p not in sys.path:
        sys.path.insert(0, _p)

N = 10000
NCORES = 8
BLK = N // NCORES          # 1250 vertices per core
HALO = 36                  # 12 (layer-0 window) + 24 (layer-1 window)
W = 1344                   # on-chip free width (1322 valid + pad)
NT = 10                    # output transpose tiles
TP = 125                   # lanes per transpose tile (10*125 = 1250 owned)
RF1, RF2 = 13, 25
C_IN, C_HID, MLP_H, MSG, NCLS = 512, 128, 128, 64, 16
LO, HI = HALO, HALO + BLK  # valid output lane range [36, 1286)
WPACK_COLS = 1676          # bf16 weights/biases/mask + fp8 fc1 + zeros
FP8_FC1 = True             # fc1 via fp8e4m3 DoubleRow (else bf16 4-pass)
FP8_SW, FP8_SX = 16.0, 8.0  # quant scales (avoid fp8 subnormals)
WARM_GROUPS = 4            # PE clock-ramp dummy groups during the DMA phase

CTS0 = ((0, 500), (500, 500), (1000, 344))       # fc1 + layer-0 col tiles
SUB13 = ((1, 488), (488, 988), (988, 1332))      # S0 = P13[+12] - P13[-1]
CTS1 = ((0, 488), (488, 500), (988, 344))        # layer-1 col tiles
SUB25 = ((1, 464), (464, 964), (964, 1286))      # S1 = P25[+24] - P25[-1]
CTS2 = ((36, 411), (411, 911), (911, 1161), (1161, 1286))  # fc2 chunks
TTILES = ((0, 3), (3, 7), (7, 9), (9, 10))       # transpose tiles per chunk

_F32 = np.float32


# --------------------------------------------------------------------------
# structure check (is the input the circulant graph the kernel was built for?)
# --------------------------------------------------------------------------

def _expected_idx():
    v = np.arange(N)
    return {
        "f_rf1": ((v[:, None] + np.arange(RF1)) % N).reshape(-1),
        "f_rf2": ((v[:, None] + np.arange(RF2)) % N).reshape(-1),
        "r_rf1": ((v[:, None] - np.arange(RF1)) % N).reshape(-1),
        "r_rf2": ((v[:, None] - np.arange(RF2)) % N).reshape(-1),
        "own1": np.repeat(v, RF1),
        "own2": np.repeat(v, RF2),
        "self1": v * RF1,
    }


def _structure_matches(inputs):
    try:
        if inputs["sparse_feature"].shape != (N, C_IN):
            return False
        for k, exp in _expected_idx().items():
            got = np.asarray(inputs[k])
            if got.shape != exp.shape or not np.array_equal(got, exp):
                return False
        return True
    except Exception:
        return False


# --------------------------------------------------------------------------
# generic numpy fallback (exact reference semantics, any index content)
# --------------------------------------------------------------------------

def _segment_sum(data, seg, num):
    out = np.zeros((num,) + data.shape[1:], dtype=data.dtype)
    np.add.at(out, seg, data)
    return out


def _np_branch(dense, rf1, rf2, own1, own2, self1, w0a, w0b, w1a, w1b):
    sizes1 = _segment_sum(np.ones(own1.shape, dense.dtype), own1, N)
    sizes2 = _segment_sum(np.ones(own2.shape, dense.dtype), own2, N)
    g = dense[rf1]
    m0 = np.concatenate([g, g], axis=-1) / sizes1[own1][:, None]
    h0 = np.maximum(np.maximum(m0 @ w0a, 0.0) @ w0b, 0.0)
    s0 = _segment_sum(h0, own1, N)
    selfr = h0[self1]
    m1 = np.concatenate([s0[rf2], selfr[rf2]], axis=-1) / sizes2[own2][:, None]
    h1 = np.maximum(np.maximum(m1 @ w1a, 0.0) @ w1b, 0.0)
    s1 = _segment_sum(h1, own2, N)
    return s0, s1


def _reference_numpy(inputs):
    f = {k: np.asarray(v) for k, v in inputs.items()}
    dense = np.maximum(
        f["sparse_feature"].astype(_F32) @ f["fc1_w"] + f["fc1_b"], 0.0
    )
    s0f, s1f = _np_branch(dense, f["f_rf1"], f["f_rf2"], f["own1"], f["own2"],
                          f["self1"], f["mw0a"], f["mw0b"], f["mw1a"], f["mw1b"])
    s0r, s1r = _np_branch(dense, f["r_rf1"], f["r_rf2"], f["own1"], f["own2"],
                          f["self1"], f["rw0a"], f["rw0b"], f["rw1a"], f["rw1b"])
    total = np.concatenate([dense, s0f, s1f, s0r, s1r], axis=1)
    logits = total @ f["fc2_w"] + f["fc2_b"]
    m = logits.max(axis=-1, keepdims=True)
    lse = m + np.log(np.exp(logits - m).sum(axis=-1, keepdims=True))
    return ((logits - lse) * f["mask"][:, None].astype(_F32)).astype(_F32)


# --------------------------------------------------------------------------
# device kernel
# --------------------------------------------------------------------------

_NC = None


def _build_nc(repeat=1, hw_loop=0):
    import concourse.bass as bass
    import concourse.tile as tile
    from concourse import bacc, mybir
    from contextlib import ExitStack

    f32 = mybir.dt.float32
    f32r = mybir.dt.float32r
    bf16 = mybir.dt.bfloat16
    fp8 = mybir.dt.float8e4
    AF = mybir.ActivationFunctionType
    OP = mybir.AluOpType
    DR = mybir.MatmulPerfMode.DoubleRow

    nc = bacc.Bacc(trn_type="TRN2", debug=False)

    xdt = fp8 if FP8_FC1 else bf16
    xt_d = nc.dram_tensor("xt", [128, 4, W], xdt, kind="ExternalInput").ap()
    wpack_d = nc.dram_tensor("wpack", [128, WPACK_COLS], bf16,
                             kind="ExternalInput").ap()
    out_d = nc.dram_tensor("out", [BLK, NCLS], f32, kind="ExternalOutput").ap()

    with tile.TileContext(nc) as tc:
        with ExitStack() as ctx:
            cp = ctx.enter_context(tc.tile_pool(name="consts", bufs=2))
            ap_ = ctx.enter_context(tc.tile_pool(name="acts", bufs=3))
            sp = ctx.enter_context(tc.tile_pool(name="scr", bufs=6))
            pmm = ctx.enter_context(tc.tile_pool(name="pmm", bufs=3, space="PSUM"))
            pzz = ctx.enter_context(tc.tile_pool(name="pzz", bufs=2, space="PSUM"))
            pl = ctx.enter_context(tc.tile_pool(name="pl", bufs=2, space="PSUM"))
            pt = ctx.enter_context(tc.tile_pool(name="pt", bufs=1, space="PSUM"))

            def emit():
                # ---- input DMAs: 3 total (dispatch costs ~1.25us each,
                # so batching beats per-tile chunking)
                xt_pack = cp.tile([128, 4, W], xdt, tag="xtp", name="xt_pack")
                wpack = cp.tile([128, WPACK_COLS], bf16, tag="wpack",
                                name="wpack")
                nc.sync.dma_start(out=wpack, in_=wpack_d)
                nc.sync.dma_start(out=xt_pack[:, 0:2, :], in_=xt_d[:, 0:2, :])
                nc.scalar.dma_start(out=xt_pack[:, 2:4, :],
                                    in_=xt_d[:, 2:4, :])

                if WARM_GROUPS:
                    # clock-ramp groups: read the (already loaded) weight
                    # pack directly - no scratch memset needed.
                    warm = pl.tile([NCLS, 512], f32, tag="psL", name="warm")
                    for i in range(WARM_GROUPS):
                        nc.tensor.matmul(warm, wpack[:, 0:NCLS],
                                         wpack[:, 0:512],
                                         start=(i == 0),
                                         stop=(i == WARM_GROUPS - 1),
                                         skip_group_check=True)

                wfc1 = [wpack[:, 128 * k:128 * (k + 1)] for k in range(4)]

                def dr_lhsT(g):
                    # fc1 fp8 weights live in wpack cols [1356+128g, 1356+
                    # 128(g+1)) (bf16 cols = 256 fp8 each); DoubleRow wants
                    # the [K, 2, M] view.
                    sl = wpack[:, 1356 + 128 * g:1356 + 128 * (g + 1)] \
                        .bitcast(fp8)
                    return bass.AP(tensor=sl.tensor, offset=sl.offset,
                                   ap=[sl.ap[0], [128, 2], [1, 128]])

                wz = {"f": wpack[:, 512:640], "r": wpack[:, 640:768]}
                wzb = {"f": wpack[:, 768:832], "r": wpack[:, 832:896]}
                # layer-1 weight halves: cols 896:1024 hold the s0 (lo) half,
                # cols 1024:1152 the z (hi) half; partitions 0:64 = forward
                # branch, 64:128 = reverse branch (matching S0/Z layout).
                wz1lo = {"f": wpack[0:64, 896:1024],
                         "r": wpack[64:128, 896:1024]}
                wz1hi = {"f": wpack[0:64, 1024:1152],
                         "r": wpack[64:128, 1024:1152]}
                wz1b = {"f": wpack[:, 1152:1216], "r": wpack[:, 1216:1280]}
                w2d = wpack[:, 1280:1296]
                w2s0 = wpack[:, 1296:1312]
                w2s1 = wpack[:, 1312:1328]
                zw64 = wpack[:, 1612:1676]    # 64 zero cols (edge zeroing)
                ident = wpack[0:16, 1328:1344]
                bfc1 = wpack[:, 1344:1345]
                bfc2 = wpack[0:16, 1345:1346]
                maskv = wpack[:, 1346:1346 + NT]

                # ---- persistent activation tiles (bf16; prefix sums f32) ----
                D = ap_.tile([128, W], bf16, tag="D")
                Z = ap_.tile([128, W], bf16, tag="Z")    # [0:64]=z_f, [64:]=z_r>>12
                Z1 = ap_.tile([128, W], bf16, tag="Z1")  # [0:64]=z1_f, [64:]=z1_r>>24
                S0 = ap_.tile([128, W], bf16, tag="S0")  # [0:64]=s0f, [64:]=s0r
                T1R = ap_.tile([128, W], bf16, tag="T1R")  # relu(t1) reverse
                T2R = ap_.tile([128, W], bf16, tag="T2R")  # relu(t2) reverse
                P13 = ap_.tile([128, W], f32, tag="P13")  # prefix sums of Z
                P25 = ap_.tile([128, W], f32, tag="P25")  # prefix sums of Z1
                S1 = ap_.tile([128, W], bf16, tag="S1")  # [0:64]=s1f, [64:]=s1r
                Lsb = ap_.tile([NCLS, W], bf16, tag="Lsb")
                LT = ap_.tile([128, NT, NCLS], f32, tag="LT")
                se = sp.tile([128, NT], f32, tag="se", name="se")
                ex = sp.tile([128, NT, NCLS], f32, tag="ex", name="ex")
                psT = pt.tile([128, NT, NCLS], bf16, tag="psT", name="psT")

                def stage_a(j):
                    """fc1 + layer-0 MLPs + chained scan + window sub."""
                    s, w = CTS0[j]
                    psA = pmm.tile([128, 512], f32, tag="mm", name="psA")
                    if FP8_FC1:
                        for g in range(2):
                            nc.tensor.matmul(psA[:, :w], dr_lhsT(g),
                                             xt_pack[:, 2 * g:2 * g + 2, s:s + w],
                                             start=(g == 0), stop=(g == 1),
                                             perf_mode=DR)
                    else:
                        for k in range(4):
                            nc.tensor.matmul(psA[:, :w], wfc1[k],
                                             xt_pack[:, k, s:s + w],
                                             start=(k == 0), stop=(k == 3))
                    nc.scalar.activation(D[:, s:s + w], psA[:, :w], AF.Relu,
                                         bias=bfc1,
                                         scale=(1.0 / (FP8_SW * FP8_SX)
                                                if FP8_FC1 else 1.0))
                    # forward branch t1
                    t1 = pmm.tile([128, 512], f32, tag="mm", name="t1")
                    nc.tensor.matmul(t1[:, :w], wz["f"], D[:, s:s + w],
                                     start=True, stop=True)
                    t1s = sp.tile([128, 512], bf16, tag="t1s", name="t1sf")
                    nc.scalar.activation(t1s[:, :w], t1[:, :w], AF.Relu)
                    # reverse branch t1 (persistent, consumed shifted by zpr)
                    t1r = pmm.tile([128, 512], f32, tag="mm", name="t1r")
                    nc.tensor.matmul(t1r[:, :w], wz["r"], D[:, s:s + w],
                                     start=True, stop=True)
                    nc.vector.tensor_scalar_max(T1R[:, s:s + w], t1r[:, :w], 0.0)
                    # both branches' z into one PSUM tile; r shifted +12 via
                    # its rhs read offset
                    psZ = pzz.tile([128, 512], f32, tag="zz", name="psZ")
                    nc.tensor.matmul(psZ[0:64, 0:w], wzb["f"], t1s[:, :w],
                                     start=True, stop=True)
                    if j == 0:
                        nc.tensor.matmul(psZ[64:128, 0:12], zw64,
                                         wpack[:, 0:12], start=True, stop=True)
                        nc.tensor.matmul(psZ[64:128, 12:w], wzb["r"],
                                         T1R[:, 0:w - 12], start=True,
                                         stop=True)
                    else:
                        nc.tensor.matmul(psZ[64:128, 0:w], wzb["r"],
                                         T1R[:, s - 12:s + w - 12],
                                         start=True, stop=True)
                    nc.scalar.activation(Z[:, s:s + w], psZ[:, 0:w], AF.Relu)
                    # chained prefix scan over both branches, then the
                    # staircase window-subtract for S0
                    nc.vector.tensor_tensor_scan(
                        P13[:, s:s + w], Z[:, s:s + w], Z[:, s:s + w],
                        initial=(0.0 if s == 0 else P13[:, s - 1:s]),
                        op0=OP.add, op1=OP.bypass)
                    lo, hi = SUB13[j]
                    nc.vector.tensor_sub(S0[:, lo:hi], P13[:, lo + 12:hi + 12],
                                         P13[:, lo - 1:hi - 1])
                    if j == 0:
                        nc.scalar.copy(S0[:, 0:1], P13[:, 12:13])

                def stage_b(j):
                    """layer-1 MLPs + chained scan + window sub (Pool)."""
                    a, w1 = CTS1[j]
                    t2 = pmm.tile([128, 512], f32, tag="mm", name="t2")
                    nc.tensor.matmul(t2[:, :w1], wz1lo["f"], S0[0:64, a:a + w1],
                                     start=True, stop=False)
                    nc.tensor.matmul(t2[:, :w1], wz1hi["f"], Z[0:64, a:a + w1],
                                     start=False, stop=True)
                    t2s = sp.tile([128, 512], bf16, tag="t1s", name="t2sf")
                    nc.scalar.activation(t2s[:, :w1], t2[:, :w1], AF.Relu)
                    t2r = pmm.tile([128, 512], f32, tag="mm", name="t2r")
                    nc.tensor.matmul(t2r[:, :w1], wz1lo["r"], S0[64:128, a:a + w1],
                                     start=True, stop=False)
                    nc.tensor.matmul(t2r[:, :w1], wz1hi["r"],
                                     Z[64:128, a + 12:a + 12 + w1],
                                     start=False, stop=True)
                    nc.vector.tensor_scalar_max(T2R[:, a:a + w1], t2r[:, :w1], 0.0)
                    psZ1 = pzz.tile([128, 512], f32, tag="zz", name="psZ1")
                    nc.tensor.matmul(psZ1[0:64, 0:w1], wz1b["f"], t2s[:, :w1],
                                     start=True, stop=True)
                    if j == 0:
                        nc.tensor.matmul(psZ1[64:128, 0:24], zw64,
                                         wpack[:, 0:24], start=True, stop=True)
                        nc.tensor.matmul(psZ1[64:128, 24:w1], wz1b["r"],
                                         T2R[:, 0:w1 - 24], start=True,
                                         stop=True)
                    else:
                        nc.tensor.matmul(psZ1[64:128, 0:w1], wz1b["r"],
                                         T2R[:, a - 24:a + w1 - 24],
                                         start=True, stop=True)
                    if j == 1:
                        nc.scalar.activation(Z1[:, a:a + w1], psZ1[:, 0:w1],
                                             AF.Relu)
                    else:
                        nc.vector.tensor_scalar_max(Z1[:, a:a + w1],
                                                    psZ1[:, 0:w1], 0.0)
                    nc.vector.tensor_tensor_scan(
                        P25[:, a:a + w1], Z1[:, a:a + w1], Z1[:, a:a + w1],
                        initial=(0.0 if a == 0 else P25[:, a - 1:a]),
                        op0=OP.add, op1=OP.bypass)
                    lo, hi = SUB25[j]
                    eng = nc.vector if j == 2 else nc.gpsimd
                    eng.tensor_sub(S1[:, lo:hi], P25[:, lo + 24:hi + 24],
                                   P25[:, lo - 1:hi - 1])

                def bcast(t2d, n):
                    return bass.AP(tensor=t2d.tensor, offset=t2d.offset,
                                   ap=[t2d.ap[0], [t2d.ap[1][0], n], [0, NCLS]])

                def stage_c(c):
                    """fc2 chunk (s1 straight from P25) + bias + transposes."""
                    lo, hi = CTS2[c]
                    w2w = hi - lo
                    psl = pl.tile([NCLS, 512], f32, tag="psL", name="psl")
                    nc.tensor.matmul(psl[:, :w2w], w2d, D[:, lo:hi],
                                     start=True, stop=False)
                    nc.tensor.matmul(psl[:, :w2w], w2s0, S0[:, lo:hi],
                                     start=False, stop=False)
                    nc.tensor.matmul(psl[:, :w2w], w2s1, S1[:, lo:hi],
                                     start=False, stop=True)
                    nc.scalar.activation(Lsb[:, lo:hi], psl[:, :w2w],
                                         AF.Identity, bias=bfc2)
                    t0, t1_ = TTILES[c]
                    for t in range(t0, t1_):
                        off = LO + TP * t
                        nc.tensor.transpose(psT[0:TP, t, :],
                                            Lsb[:, off:off + TP], ident)

                def softmax_out(t0, t1_, c):
                    """log-softmax + mask + output DMA for transpose tiles
                    [t0, t1).  Logits are bounded (|L| ~ 2): exp without
                    max-subtract."""
                    nt = t1_ - t0
                    lt = LT[0:TP, t0:t1_, :]
                    ps3 = psT[0:TP, t0:t1_, :]
                    seh = se[0:TP, t0:t1_]
                    if nt == 1:
                        # single tile: exp's accumulator IS the class sum
                        nc.scalar.activation(ex[0:TP, t0:t1_, :], ps3, AF.Exp,
                                             accum_out=seh)
                    else:
                        nc.scalar.activation(ex[0:TP, t0:t1_, :], ps3, AF.Exp)
                        nc.vector.reduce_sum(seh, ex[0:TP, t0:t1_, :],
                                             axis=mybir.AxisListType.X)
                    nc.scalar.activation(seh, seh, AF.Ln)
                    nc.vector.tensor_sub(lt, ps3, bcast(seh, nt))
                    mm = nc.vector if nt == 1 else nc.gpsimd
                    mm.tensor_mul(lt, lt, bcast(maskv[0:TP, t0:t1_], nt))

                # interleave stages so PE never starves on eviction chains
                stage_a(0)
                stage_a(1)
                stage_b(0)
                stage_a(2)
                stage_b(1)
                stage_c(0)
                stage_b(2)
                stage_c(1)
                stage_c(2)
                softmax_out(0, 9, 0)
                od = out_d.rearrange("(t p) c -> p t c", p=TP)
                nc.sync.dma_start(out=od[:, 0:9, :], in_=LT[0:TP, 0:9, :])
                stage_c(3)
                softmax_out(9, 10, 1)
                nc.scalar.dma_start(out=od[:, 9:10, :], in_=LT[0:TP, 9:10, :])

            if hw_loop:
                unroll = 32 if hw_loop % 32 == 0 else 2
                assert hw_loop % unroll == 0
                with tc.For_i(0, hw_loop // unroll):
                    for _u in range(unroll):
                        emit()
            else:
                for _rep in range(repeat):
                    emit()

    # Steer the ACT-table pass to natural_log_exp_and_others (covers Relu,
    # Identity, Copy, Exp AND Ln) so the kernel pays one table load instead
    # of a ~2.7us mid-kernel switch before the final Ln.
    import concourse.bacc as bacc_mod
    from concourse import mybir as _mb

    AF = _mb.ActivationFunctionType
    orig_tables = bacc_mod.get_activation_tables
    mine = {AF.Relu, AF.Identity, AF.Copy, AF.Exp, AF.Ln}

    def steered(arch):
        t = orig_tables(arch)
        out = {}
        seen_pref = False
        for name, fns in t.items():
            if name == "natural_log_exp_and_others":
                seen_pref = True
                out[name] = fns
            elif not seen_pref:
                out[name] = type(fns)(f for f in fns if f not in mine)
            else:
                out[name] = fns
        return out

    bacc_mod.get_activation_tables = steered
    try:
        nc.compile()
    finally:
        bacc_mod.get_activation_tables = orig_tables
    return nc


def _get_nc(repeat=1, hw_loop=0):
    global _NC
    if repeat != 1 or hw_loop:
        return _build_nc(repeat, hw_loop)
    if _NC is None:
        _NC = _build_nc()
    return _NC


# --------------------------------------------------------------------------
# host-side sharding + entry point
# --------------------------------------------------------------------------

def _make_in_maps(inputs):
    from concourse import mybir

    bf16np = mybir.dt.np(mybir.dt.bfloat16)
    fp8np = mybir.dt.np(mybir.dt.float8e4)
    sf = np.ascontiguousarray(np.asarray(inputs["sparse_feature"], dtype=_F32))
    maskf = np.asarray(inputs["mask"]).astype(_F32)

    def f(k):
        return np.asarray(inputs[k], dtype=_F32)

    mw0a, rw0a = f("mw0a"), f("rw0a")
    wpack = np.zeros((128, WPACK_COLS), dtype=_F32)
    wpack[:, 0:512] = f("fc1_w").reshape(4, 128, C_HID).transpose(1, 0, 2) \
        .reshape(128, 512)
    wpack[:, 512:640] = (mw0a[:C_HID] + mw0a[C_HID:]) / RF1
    wpack[:, 640:768] = (rw0a[:C_HID] + rw0a[C_HID:]) / RF1
    wpack[:, 768:832] = f("mw0b")
    wpack[:, 832:896] = f("rw0b")
    mw1a, rw1a = f("mw1a") / RF2, f("rw1a") / RF2
    wpack[0:64, 896:1024] = mw1a[0:64]      # s0 half, forward
    wpack[64:128, 896:1024] = rw1a[0:64]    # s0 half, reverse
    wpack[0:64, 1024:1152] = mw1a[64:128]   # z half, forward
    wpack[64:128, 1024:1152] = rw1a[64:128]  # z half, reverse
    wpack[:, 1152:1216] = f("mw1b")
    wpack[:, 1216:1280] = f("rw1b")
    w2 = f("fc2_w")
    wpack[:, 1280:1296] = w2[0:128]         # dense
    wpack[0:64, 1296:1312] = w2[128:192]    # s0f
    wpack[64:128, 1296:1312] = w2[256:320]  # s0r
    wpack[0:64, 1312:1328] = w2[192:256]    # s1f
    wpack[64:128, 1312:1328] = w2[320:384]  # s1r
    wpack[0:16, 1328:1344] = np.eye(NCLS, dtype=_F32)
    wpack[:, 1344] = f("fc1_b")
    wpack[0:NCLS, 1345] = f("fc2_b")

    # fc1 weights as [128, k, 128] blocks (k = input-channel block),
    # scaled into fp8's normal range (descaled in the D eviction)
    wf8 = np.ascontiguousarray(
        f("fc1_w").reshape(4, 128, C_HID).transpose(1, 0, 2)
        * FP8_SW).astype(fp8np)

    in_maps = []
    for c in range(NCORES):
        b = c * BLK
        idx = (b - HALO + np.arange(W)) % N
        xt2 = np.ascontiguousarray(sf[idx].T)          # [512, W]
        if FP8_FC1:
            xt2 = xt2 * FP8_SX
        xt = np.ascontiguousarray(
            xt2.reshape(4, 128, W).transpose(1, 0, 2))  # [128, 4, W]
        me = np.zeros(128 * NT, dtype=_F32)
        me[:BLK] = maskf[(b + np.arange(BLK)) % N]
        wc = wpack.copy()
        wc[0:TP, 1346:1346 + NT] = me[:BLK].reshape(NT, TP).T
        wcb = wc.astype(bf16np)
        if FP8_FC1:
            wcb.view(np.uint8).reshape(128, 2 * WPACK_COLS)[:, 2712:3224] = \
                wf8.view(np.uint8).reshape(128, 512)
        m = {"wpack": wcb,
             "xt": xt.astype(fp8np if FP8_FC1 else bf16np)}
        in_maps.append(m)
    return in_maps


_RUNNER = None


def _make_runner():
    """Build the 8-core PJRT executor once; reuse across kernel() calls."""
    import jax
    from jax.sharding import Mesh, NamedSharding, PartitionSpec
    from jax.experimental.shard_map import shard_map
    from concourse import mybir
    from concourse.bass2jax import (_bass_exec_p, install_neuronx_cc_hook,
                                    partition_id_tensor)

    nc = _get_nc()
    install_neuronx_cc_hook()
    in_names, out_names, out_avals, zero_shapes = [], [], [], []
    pname = nc.partition_id_tensor.name if nc.partition_id_tensor else None
    for alloc in nc.m.functions[0].allocations:
        if not isinstance(alloc, mybir.MemoryLocationSet):
            continue
        name = alloc.memorylocations[0].name
        if alloc.kind == "ExternalInput":
            if name != pname:
                in_names.append(name)
        elif alloc.kind == "ExternalOutput":
            out_names.append(name)
            shape = tuple(alloc.tensor_shape)
            dtype = mybir.dt.np(alloc.dtype)
            out_avals.append(jax.core.ShapedArray(shape, dtype))
            zero_shapes.append((shape, dtype))
    n_params = len(in_names)
    all_in = list(in_names) + list(out_names)
    if pname is not None:
        all_in.append(pname)
    donate = tuple(range(n_params, n_params + len(out_names)))

    def _body(*args):
        operands = list(args)
        if pname is not None:
            operands.append(partition_id_tensor())
        return tuple(_bass_exec_p.bind(
            *operands,
            out_avals=tuple(out_avals),
            in_names=tuple(all_in),
            out_names=tuple(out_names),
            lowering_input_output_aliases=(),
            sim_require_finite=True,
            sim_require_nnan=True,
            nc=nc,
        ))

    devices = jax.devices()[:NCORES]
    mesh = Mesh(np.asarray(devices), ("core",))
    shd = NamedSharding(mesh, PartitionSpec("core"))
    n_outs = len(out_names)
    sharded = jax.jit(
        shard_map(_body, mesh=mesh,
                  in_specs=(PartitionSpec("core"),) * (n_params + n_outs),
                  out_specs=(PartitionSpec("core"),) * n_outs,
                  check_rep=False),
        donate_argnums=donate, keep_unused=True,
    )

    def run(in_maps):
        concat_in = [
            np.concatenate([np.asarray(in_maps[c][nm]) for c in range(NCORES)],
                           axis=0)
            for nm in in_names
        ]
        dev_in = [jax.device_put(x, shd) for x in concat_in]
        zeros = [
            jax.device_put(np.zeros((NCORES * s[0], *s[1:]), dt), shd)
            for s, dt in zero_shapes
        ]
        outs = sharded(*dev_in, *zeros)
        res = np.asarray(outs[out_names.index("out")])
        return np.ascontiguousarray(res.reshape(NCORES * BLK, NCLS))

    return run


def kernel(**inputs):
    if not _structure_matches(inputs):
        return _reference_numpy(inputs)
    global _RUNNER
    if _RUNNER is None:
        _RUNNER = _make_runner()
    return _RUNNER(_make_in_maps(inputs))
